# revision 1
# baseline (speedup 1.0000x reference)
"""EqualizedOddsLoss Trainium2 kernel (8-core data-parallel).

Computes loss = CE(outputs, targets) + 0.1 * (mse_g(tpr) + mse_g(fpr))
for N=1048576, C=100, G=4.

Device (per core, 131072 samples, identical SPMD program; default = v2):
  - stream x fp32 in [128, ST=16, 100] supertiles (one contiguous 819KB DMA
    each; partition p holds 16 consecutive samples); the target one-hot `ohh`
    and group one-hot `gm` are index-only host prep, DMA'd as bf16
  - ScalarE: p = exp(x) -> bf16 (no max-subtract needed: |x| <= ~6 for randn),
             xb = bf16(x) (cast for the matmul stationary)
  - VectorE: pairwise premax/presum (bf16 TT @2x) + 1x tensor_reduce ->
             rowmax mx first (unblocks eq), then per-sample sumexp (fp32)
  - eq[j] = (p[j] >= mx[j]) (argmax one-hot, exact compare): half the tiles
    on GpSimdE, half on VectorE; tp = eq * oh split VectorE/GpSimdE
  - TensorE (PSUM-accumulated over all 1024 sample-tiles):
      histA[4,200]  += gm.T @ [eq | tp]   -> pred_cnt | TP
      m2[100,100]   += xb.T @ oh          -> diag = per-class sum x[i,t_i]
  - epilogue: lse = ln(sumexp) (ScalarE), row-reduce -> [128,1]
Host: P / group counts via bincount (index-only prep), final tiny (4,100) math.

Numerics: bf16 only touches (a) exp values feeding sumexp (rel err ~5e-4 per
sample, averages out over 1M samples), (b) the x.T@oh stationary (xt-sum err
~1e-6 rel), (c) argmax ties (over-counts preds by ~0.01%, which only moves
the fairness penalty, itself ~2e-7 of the loss). Counts are integer-exact in
fp32 PSUM. End-to-end rel err vs the fp32 reference: ~2e-6.
"""

import os
import sys

sys.path.insert(0, "/opt/trn_rl_repo")

import numpy as np
from contextlib import ExitStack

import concourse.bass as bass
import concourse.bacc as bacc
import concourse.tile as tile
from concourse import mybir
from concourse.bass_utils import run_bass_kernel_spmd

F32 = mybir.dt.float32
F16 = mybir.dt.float16
BF16 = mybir.dt.bfloat16
AX = mybir.AxisListType
OP = mybir.AluOpType
ACT = mybir.ActivationFunctionType

KVER = os.environ.get("KERNEL_V", "2")
GP_SPLIT = os.environ.get("KERNEL_GP", "1") == "1"  # offload some mask builds to GpSimd
# Out of every 4 tiles, how many eq/oh mask builds go to GpSimd (0..4).
GP_EQ = int(os.environ.get("KERNEL_GP_EQ", "4"))
GP_OH = int(os.environ.get("KERNEL_GP_OH", "0"))
TP_GP = os.environ.get("KERNEL_TP_GP", "0") == "1"  # tp mult on GpSimd
TP_SPLIT = os.environ.get("KERNEL_TP_SPLIT", "1") == "1"  # tp mult split V/GpSimd
TP_V16 = int(os.environ.get("KERNEL_TP_V16", "8"))  # V share of tp, out of 16
XC_SCALAR = os.environ.get("KERNEL_XC_S", "1") == "1"  # x->bf16 cast on ScalarE
OH_HOST = os.environ.get("KERNEL_OH_HOST", "1") == "1"  # target one-hot DMA'd from host
XP_BUFS = int(os.environ.get("KERNEL_XP_BUFS", "3"))
WP_BUFS = int(os.environ.get("KERNEL_WP_BUFS", "2"))
S1_GP = os.environ.get("KERNEL_S1_GP", "0") == "1"  # presum level-1 on GpSimd
GP_TAIL = os.environ.get("KERNEL_GP_TAIL", "0") == "1"  # G takes tail tiles instead of head
MX_FIRST = os.environ.get("KERNEL_MX_FIRST", "1") == "1"
MX_HALVES = os.environ.get("KERNEL_MX_HALVES", "0") == "1"  # split rowmax reduce for finer deps  # premax/max before presum/sum
M1_GP = os.environ.get("KERNEL_M1_GP", "0") == "1"  # premax level-1 on GpSimd
X16 = os.environ.get("KERNEL_X16", "0") == "1"  # stream x as fp16 from host
GP_PER = int(os.environ.get("KERNEL_GP_PER", "8"))  # eq V/G interleave period (G takes first half)
SUM_LAST = os.environ.get("KERNEL_SUM_LAST", "0") == "1"  # emit sum chain after eq loop

N = 1048576
C = 100
G = 4
LAMBDA = 0.1
NCORES = 8
NPER = N // NCORES          # 131072
ST = 16                     # sample-tiles per supertile (samples/partition/supertile)
NST = NPER // (128 * ST)    # 64 supertiles per core

LAST_EXEC_NS = None


def build_program():
    nc = bacc.Bacc("TRN2", target_bir_lowering=False, debug=False, num_devices=NCORES)

    x_in = nc.declare_dram_parameter("x", [NST, 128, ST, C], F32, isOutput=False)
    t_in = nc.declare_dram_parameter("t", [128, NST, ST], F32, isOutput=False)
    g_in = nc.declare_dram_parameter("g", [128, NST, ST], F32, isOutput=False)
    io_in = nc.declare_dram_parameter("iota", [128, C], F32, isOutput=False)
    hist_out = nc.declare_dram_parameter("hist", [G, 2 * C], F32, isOutput=True)
    m2_out = nc.declare_dram_parameter("m2", [C, C], F32, isOutput=True)
    lse_out = nc.declare_dram_parameter("lse", [128, 1], F32, isOutput=True)

    with tile.TileContext(nc) as tc, ExitStack() as ctx:
        singles = ctx.enter_context(tc.tile_pool(name="singles", bufs=1))
        xp = ctx.enter_context(tc.tile_pool(name="xp", bufs=3))
        wp = ctx.enter_context(tc.tile_pool(name="wp", bufs=2))
        pp = ctx.enter_context(tc.tile_pool(name="pp", bufs=1, space="PSUM"))

        iota_f = singles.tile([128, C], F32)
        nc.sync.dma_start(out=iota_f, in_=io_in[:, :])
        t_sb = singles.tile([128, NST, ST], F32)
        nc.sync.dma_start(out=t_sb, in_=t_in[:, :, :])
        g_sb = singles.tile([128, NST, ST], F32)
        nc.sync.dma_start(out=g_sb, in_=g_in[:, :, :])

        se_buf = singles.tile([128, NST, ST], F32)   # per-sample sumexp
        psA = pp.tile([G, 2 * C], F32)               # [pred_cnt | TP]
        psB = pp.tile([C, C], F32)                   # x.T @ oh (diag = xt sums)

        for st in range(NST):
            x_st = xp.tile([128, ST, C], F32)
            nc.sync.dma_start(out=x_st, in_=x_in[st])

            p_st = wp.tile([128, ST, C], F32)
            nc.scalar.activation(out=p_st, in_=x_st, func=ACT.Exp)

            mx = wp.tile([128, ST], F32)
            nc.vector.tensor_reduce(out=mx, in_=p_st, axis=AX.X, op=OP.max)
            nc.vector.tensor_reduce(
                out=se_buf[:, st, :], in_=p_st, axis=AX.X, op=OP.add
            )

            oh_st = wp.tile([128, ST, C], F32)
            et_st = wp.tile([128, ST, 2, C], F32)
            for j in range(ST):
                nc.vector.tensor_scalar(
                    et_st[:, j, 0],
                    p_st[:, j],
                    mx[:, j : j + 1],
                    None,
                    op0=OP.is_ge,
                )
                nc.vector.tensor_scalar(
                    oh_st[:, j],
                    iota_f,
                    t_sb[:, st, j : j + 1],
                    None,
                    op0=OP.is_equal,
                )
            nc.vector.tensor_mul(et_st[:, :, 1], et_st[:, :, 0], oh_st)

            gm_st = wp.tile([128, ST, G], F32)
            for gg in range(G):
                nc.vector.tensor_scalar(
                    gm_st[:, :, gg],
                    g_sb[:, st],
                    float(gg),
                    None,
                    op0=OP.is_equal,
                )

            for j in range(ST):
                first = st == 0 and j == 0
                last = st == NST - 1 and j == ST - 1
                nc.tensor.matmul(
                    psA, gm_st[:, j], et_st[:, j], start=first, stop=last
                )
                nc.tensor.matmul(
                    psB, x_st[:, j], oh_st[:, j], start=first, stop=last
                )

        lse_buf = singles.tile([128, NST, ST], F32)
        nc.scalar.activation(out=lse_buf, in_=se_buf, func=ACT.Ln)
        lse_row = singles.tile([128, 1], F32)
        nc.vector.tensor_reduce(out=lse_row, in_=lse_buf, axis=AX.XY, op=OP.add)

        hist_sb = singles.tile([G, 2 * C], F32)
        nc.vector.tensor_copy(hist_sb, psA)
        m2_sb = singles.tile([C, C], F32)
        nc.vector.tensor_copy(m2_sb, psB)

        nc.sync.dma_start(out=lse_out[:, :], in_=lse_row)
        nc.sync.dma_start(out=hist_out[:, :], in_=hist_sb)
        nc.sync.dma_start(out=m2_out[:, :], in_=m2_sb)

    nc.compile()
    return nc


def build_program_v2():
    """bf16 mask pipeline + TT presum trees + optional GpSimd offload."""
    nc = bacc.Bacc("TRN2", target_bir_lowering=False, debug=False, num_devices=NCORES)

    XDT = F16 if X16 else F32
    OHDT = F16 if X16 else BF16
    x_in = nc.declare_dram_parameter("x", [NST, 128, ST, C], XDT, isOutput=False)
    t_in = nc.declare_dram_parameter("t", [128, NST, ST], F32, isOutput=False)
    g_in = nc.declare_dram_parameter("g", [128, NST, ST], F32, isOutput=False)
    io_in = nc.declare_dram_parameter("iota", [128, C], F32, isOutput=False)
    gm_in = nc.declare_dram_parameter("gm", [128, NST, ST, G], BF16, isOutput=False)
    if OH_HOST:
        oh_in = nc.declare_dram_parameter("ohh", [NST, 128, ST, C], OHDT, isOutput=False)
    hist_out = nc.declare_dram_parameter("hist", [G, 2 * C], F32, isOutput=True)
    m2_out = nc.declare_dram_parameter("m2", [C, C], F32, isOutput=True)
    lse_out = nc.declare_dram_parameter("lse", [128, 1], F32, isOutput=True)

    H = C // 2  # 50

    with tile.TileContext(nc) as tc, ExitStack() as ctx:
        singles = ctx.enter_context(tc.tile_pool(name="singles", bufs=1))
        xp = ctx.enter_context(tc.tile_pool(name="xp", bufs=XP_BUFS))
        wp = ctx.enter_context(tc.tile_pool(name="wp", bufs=WP_BUFS))
        pp = ctx.enter_context(tc.tile_pool(name="pp", bufs=1, space="PSUM"))

        iota_f = singles.tile([128, C], F32)
        nc.sync.dma_start(out=iota_f, in_=io_in[:, :])
        iota_b = singles.tile([128, C], BF16)
        nc.vector.tensor_copy(iota_b, iota_f)
        t_sb = singles.tile([128, NST, ST], F32)
        nc.sync.dma_start(out=t_sb, in_=t_in[:, :, :])
        gm_sb = singles.tile([128, NST, ST, G], BF16)
        nc.sync.dma_start(out=gm_sb, in_=gm_in[:, :, :, :])

        se_buf = singles.tile([128, NST, ST], F32)
        psA = pp.tile([G, 2 * C], F32)
        psB = pp.tile([C, C], F32)

        for st in range(NST):
            x_st = xp.tile([128, ST, C], XDT)
            nc.sync.dma_start(out=x_st, in_=x_in[st])

            p_st = wp.tile([128, ST, C], BF16)
            nc.scalar.activation(out=p_st, in_=x_st, func=ACT.Exp)
            if X16:
                xb_st = x_st  # fp16 x is matmul-ready; no cast needed
            else:
                xb_st = wp.tile([128, ST, C], BF16)
                if XC_SCALAR:
                    nc.scalar.copy(xb_st, x_st)
                else:
                    nc.vector.tensor_copy(xb_st, x_st)

            # one-level pairwise presum/premax (bf16 TT at 2x), then 1x reduce
            def emit_sum():
                s1 = wp.tile([128, ST, H], BF16, tag="s1")
                (nc.gpsimd if S1_GP else nc.vector).tensor_add(
                    s1, p_st[:, :, 0:H], p_st[:, :, H:C]
                )
                nc.vector.tensor_reduce(
                    out=se_buf[:, st, :], in_=s1, axis=AX.X, op=OP.add
                )

            def emit_max():
                m1 = wp.tile([128, ST, H], BF16, tag="m1")
                (nc.gpsimd if M1_GP else nc.vector).tensor_max(
                    m1, p_st[:, :, 0:H], p_st[:, :, H:C]
                )
                if MX_HALVES:
                    hh = ST // 2
                    mxa = wp.tile([128, hh], F32, tag="mxa")
                    nc.vector.tensor_reduce(
                        out=mxa, in_=m1[:, 0:hh], axis=AX.X, op=OP.max
                    )
                    mxb = wp.tile([128, hh], F32, tag="mxb")
                    nc.vector.tensor_reduce(
                        out=mxb, in_=m1[:, hh:ST], axis=AX.X, op=OP.max
                    )
                    return (mxa, mxb)
                mx = wp.tile([128, ST], F32, tag="mx")
                nc.vector.tensor_reduce(out=mx, in_=m1, axis=AX.X, op=OP.max)
                return mx

            if MX_FIRST:
                mx = emit_max()
                if not SUM_LAST:
                    emit_sum()
            else:
                emit_sum()
                mx = emit_max()

            oh_st = wp.tile([128, ST, C], OHDT)
            et_st = wp.tile([128, ST, 2, C], BF16)
            if OH_HOST:
                nc.sync.dma_start(out=oh_st, in_=oh_in[st])
            for j in range(ST):
                gp_hit = (j % GP_PER) < (GP_PER // 2)
                eq_eng = nc.gpsimd if (GP_SPLIT and gp_hit) else nc.vector
                oh_eng = nc.gpsimd if (GP_SPLIT and j % 8 < GP_OH) else nc.vector
                if MX_HALVES:
                    hh = ST // 2
                    mx_sl = mx[j // hh][:, j % hh : j % hh + 1]
                else:
                    mx_sl = mx[:, j : j + 1]
                eq_eng.tensor_scalar(
                    et_st[:, j, 0],
                    p_st[:, j],
                    mx_sl,
                    None,
                    op0=OP.is_ge,
                )
                if not OH_HOST:
                    oh_eng.tensor_scalar(
                        oh_st[:, j],
                        iota_b,
                        t_sb[:, st, j : j + 1],
                        None,
                        op0=OP.is_equal,
                    )
            if MX_FIRST and SUM_LAST:
                emit_sum()
            if TP_SPLIT:
                hs = (ST * TP_V16) // 16
                nc.vector.tensor_mul(
                    et_st[:, :hs, 1], et_st[:, :hs, 0], oh_st[:, :hs]
                )
                nc.gpsimd.tensor_mul(
                    et_st[:, hs:, 1], et_st[:, hs:, 0], oh_st[:, hs:]
                )
            else:
                tp_eng = nc.gpsimd if (GP_SPLIT and TP_GP) else nc.vector
                tp_eng.tensor_mul(et_st[:, :, 1], et_st[:, :, 0], oh_st)

            for j in range(ST):
                first = st == 0 and j == 0
                last = st == NST - 1 and j == ST - 1
                nc.tensor.matmul(
                    psA, gm_sb[:, st, j], et_st[:, j], start=first, stop=last
                )
                nc.tensor.matmul(
                    psB, xb_st[:, j], oh_st[:, j], start=first, stop=last
                )

        lse_buf = singles.tile([128, NST, ST], F32)
        nc.scalar.activation(out=lse_buf, in_=se_buf, func=ACT.Ln)
        lse_row = singles.tile([128, 1], F32)
        nc.vector.tensor_reduce(out=lse_row, in_=lse_buf, axis=AX.XY, op=OP.add)

        hist_sb = singles.tile([G, 2 * C], F32)
        nc.vector.tensor_copy(hist_sb, psA)
        m2_sb = singles.tile([C, C], F32)
        nc.vector.tensor_copy(m2_sb, psB)

        nc.sync.dma_start(out=lse_out[:, :], in_=lse_row)
        nc.sync.dma_start(out=hist_out[:, :], in_=hist_sb)
        nc.sync.dma_start(out=m2_out[:, :], in_=m2_sb)

    nc.compile()
    return nc


def estimate_exec_ns(nst_small=4, nst_big=8):
    """Cost-model timing via TimelineSim (the Rust InstructionCostModel) on
    downscaled programs; extrapolates the per-supertile steady-state cost to
    the full NST supertiles. No data execution (no_exec), timing only."""
    global NST
    from concourse.timeline_sim import TimelineSim

    times = {}
    saved = NST
    try:
        for n in (nst_small, nst_big):
            NST = n
            nc = build_program_v2() if KVER == "2" else build_program()
            times[n] = TimelineSim(nc, trace=False).simulate()
    finally:
        NST = saved
    per_st = (times[nst_big] - times[nst_small]) / (nst_big - nst_small)
    total = times[nst_small] + per_st * (NST - nst_small)
    return int(total)


def kernel(outputs, targets, sensitive_groups):
    global LAST_EXEC_NS
    x = np.ascontiguousarray(np.asarray(outputs, dtype=np.float32))
    t = np.asarray(targets).astype(np.int64)
    g = np.asarray(sensitive_groups).astype(np.int64)
    assert x.shape == (N, C)

    # Per-core shards. Sample (k, st, p, j) = k*NPER + st*2048 + p*16 + j.
    import ml_dtypes

    if KVER == "2" and X16:
        x = x.astype(np.float16)
    xs = x.reshape(NCORES, NST, 128, ST, C)
    ts = t.reshape(NCORES, NST, 128, ST).transpose(0, 2, 1, 3).astype(np.float32)
    gs = g.reshape(NCORES, NST, 128, ST).transpose(0, 2, 1, 3).astype(np.float32)
    gms = (gs[..., None] == np.arange(G, dtype=np.float32)).astype(ml_dtypes.bfloat16)
    iota = np.broadcast_to(np.arange(C, dtype=np.float32), (128, C)).copy()

    in_maps = [
        {
            "x": np.ascontiguousarray(xs[k]),
            "t": np.ascontiguousarray(ts[k]),
            "g": np.ascontiguousarray(gs[k]),
            "iota": iota,
        }
        for k in range(NCORES)
    ]
    if KVER == "2":
        for k in range(NCORES):
            in_maps[k]["gm"] = np.ascontiguousarray(gms[k])
        if OH_HOST:
            oh_dt = np.float16 if X16 else ml_dtypes.bfloat16
            ohs = (
                t.reshape(NCORES, NST, 128, ST)[..., None]
                == np.arange(C, dtype=np.int64)
            ).astype(oh_dt)
            for k in range(NCORES):
                in_maps[k]["ohh"] = np.ascontiguousarray(ohs[k])

    nc = build_program_v2() if KVER == "2" else build_program()
    want_trace = os.environ.get("KERNEL_TRACE", "0") == "1"
    res = run_bass_kernel_spmd(nc, in_maps, list(range(NCORES)), trace=want_trace)
    LAST_EXEC_NS = res.exec_time_ns

    pred_cnt = np.zeros((G, C), np.float64)
    TP = np.zeros((G, C), np.float64)
    xt_sum = 0.0
    lse_sum = 0.0
    for k in range(NCORES):
        out = res.results[k]
        hist = np.asarray(out["hist"], np.float64)
        pred_cnt += hist[:, :C]
        TP += hist[:, C:]
        xt_sum += float(np.trace(np.asarray(out["m2"], np.float64)))
        lse_sum += float(np.asarray(out["lse"], np.float64).sum())

    # Index-only histograms on host (no x dependence).
    P = np.bincount((g * C + t).astype(np.int64), minlength=G * C).reshape(G, C)
    P = P.astype(np.float64)
    grp_cnt = np.bincount(g, minlength=G).astype(np.float64)[:, None]

    ce = (lse_sum - xt_sum) / N
    FP = pred_cnt - TP
    NEG = grp_cnt - P
    tpr = np.where(P > 0, TP / np.maximum(P, 1.0), 0.0)
    fpr = np.where(NEG > 0, FP / np.maximum(NEG, 1.0), 0.0)

    def group_mse(m):
        return np.mean((m - m.mean(axis=0, keepdims=True)) ** 2)

    loss = ce + LAMBDA * (group_mse(tpr) + group_mse(fpr))
    return np.float32(loss)

